# revision 7
# baseline (speedup 1.0000x reference)
"""ArcFace (AngularPenaltySMLoss) over [32768, 8192] f32, distributed over
8 TRN2 NeuronCores, data-parallel on the batch dim.

Per core: shard [4096, 8192]. For each 128-row tile:
  - DMA tile to SBUF
  - ScalarE: exp(S*x) with fused free-dim accumulation -> row exp-sums
  - VectorE: scalar_tensor_tensor (iota == label) * x with fused free-dim
    accumulation -> gathers target = x[row, label] (one nonzero per row)
Epilogue (on [128, 32] stats):
  numerator = S*(t_clip*cos(M) - sin(M)*sqrt(1 - t_clip^2))   # = S*cos(acos(t)+M)
  L = numerator - log(exp(numerator) + rowsum - exp(S*t))
  partial = sum(L) per core -> [1,1]; host sums 8 partials, loss = -total/N.
"""

import numpy as np

from concourse import bacc, mybir, tile
from concourse.bass_utils import run_bass_kernel_spmd

N, C = 32768, 8192
N_CORES = 8
N_SHARD = N // N_CORES      # 4096 rows per core
P = 128                     # SBUF partitions
N_TILES = N_SHARD // P      # 32 tiles per core
S = 32.0
M = 0.5
EPS = 1e-7
FLT_MIN = float(np.finfo(np.float32).min)

_F32 = mybir.dt.float32


def build(n_shard=N_SHARD, c=C):
    n_tiles = n_shard // P
    nc = bacc.Bacc(None, target_bir_lowering=False)

    x_ext = nc.declare_dram_parameter("cls_score", [n_shard, c], _F32, isOutput=False)
    lab_ext = nc.declare_dram_parameter("labels_t", [P, n_tiles], _F32, isOutput=False)
    out_ext = nc.declare_dram_parameter("out", [P, 1], _F32, isOutput=True)

    AF = mybir.ActivationFunctionType
    OP = mybir.AluOpType
    AX = mybir.AxisListType

    with tile.TileContext(nc) as tc:
        with (
            tc.tile_pool(name="xp", bufs=2) as xp,
            tc.tile_pool(name="ep", bufs=1) as ep,
            tc.tile_pool(name="mp", bufs=1) as mp,
            tc.tile_pool(name="st", bufs=1) as st,
        ):
            lab = st.tile([P, n_tiles], _F32)
            nc.sync.dma_start(out=lab[:], in_=lab_ext[:])
            iota = st.tile([P, c], _F32)  # each row = [0..c-1]
            nc.gpsimd.iota(iota[:], pattern=[[1, c]], base=0,
                           channel_multiplier=0,
                           allow_small_or_imprecise_dtypes=True)

            sumexp = st.tile([P, n_tiles], _F32)
            tvals = st.tile([P, n_tiles], _F32)

            for k in range(n_tiles):
                xt = xp.tile([P, c], _F32)
                nc.sync.dma_start(out=xt[:], in_=x_ext[k * P:(k + 1) * P, :])
                et = ep.tile([P, c], _F32)
                nc.scalar.activation(
                    out=et[:], in_=xt[:], func=AF.Exp, scale=S,
                    accum_out=sumexp[:, k:k + 1],
                )
                mt = mp.tile([P, c], _F32)
                # mt = (iota == label) * x ; accum -> x[row, label]
                nc.vector.scalar_tensor_tensor(
                    mt[:], iota[:], lab[:, k:k + 1], xt[:],
                    OP.is_equal, OP.mult,
                    accum_out=tvals[:, k:k + 1],
                )

            # ---- epilogue on [P, n_tiles] ----
            tclip = st.tile([P, n_tiles], _F32)
            nc.vector.tensor_scalar(
                tclip[:], tvals[:], -1.0 + EPS, 1.0 - EPS, OP.max, OP.min
            )
            tsq = st.tile([P, n_tiles], _F32)
            nc.scalar.activation(out=tsq[:], in_=tclip[:], func=AF.Square)
            r = st.tile([P, n_tiles], _F32)  # sqrt(1 - t^2) = sin(acos(t))
            nc.scalar.activation(
                out=r[:], in_=tsq[:], func=AF.Sqrt, scale=-1.0, bias=1.0
            )
            b_t = st.tile([P, n_tiles], _F32)
            nc.vector.tensor_scalar_mul(b_t[:], r[:], S * float(np.sin(M)))
            num = st.tile([P, n_tiles], _F32)
            # num = (tclip * S*cos(M)) - b_t
            nc.vector.scalar_tensor_tensor(
                num[:], tclip[:], S * float(np.cos(M)), b_t[:], OP.mult, OP.subtract
            )
            e_num = st.tile([P, n_tiles], _F32)
            nc.scalar.activation(out=e_num[:], in_=num[:], func=AF.Exp)
            e_st = st.tile([P, n_tiles], _F32)
            nc.scalar.activation(out=e_st[:], in_=tvals[:], func=AF.Exp, scale=S)
            excl = st.tile([P, n_tiles], _F32)
            # excl = (e_st * -1) + sumexp
            nc.vector.scalar_tensor_tensor(
                excl[:], e_st[:], -1.0, sumexp[:], OP.mult, OP.add
            )
            denom = st.tile([P, n_tiles], _F32)
            nc.vector.tensor_tensor(denom[:], excl[:], e_num[:], OP.add)
            logd = st.tile([P, n_tiles], _F32)
            nc.scalar.activation(out=logd[:], in_=denom[:], func=AF.Ln)
            ell = st.tile([P, n_tiles], _F32)
            nc.vector.tensor_tensor(ell[:], num[:], logd[:], OP.subtract)
            lrow = st.tile([P, 1], _F32)
            nc.vector.tensor_reduce(lrow[:], ell[:], axis=AX.X, op=OP.add)
            nc.sync.dma_start(out=out_ext[:], in_=lrow[:])

    nc.finalize()
    return nc


_NC_CACHE = {}


def _get_nc():
    if "nc" not in _NC_CACHE:
        _NC_CACHE["nc"] = build()
    return _NC_CACHE["nc"]


def make_in_maps(cls_score, labels):
    cls_score = np.ascontiguousarray(np.asarray(cls_score, dtype=np.float32))
    labels = np.asarray(labels).astype(np.int64)
    in_maps = []
    for i in range(N_CORES):
        shard = cls_score[i * N_SHARD:(i + 1) * N_SHARD]
        lab_i = labels[i * N_SHARD:(i + 1) * N_SHARD].astype(np.float32)
        # [n_tiles, P] -> [P, n_tiles]: partition p, col k = label of row k*P+p
        lab_t = np.ascontiguousarray(lab_i.reshape(N_TILES, P).T)
        in_maps.append({"cls_score": shard, "labels_t": lab_t})
    return in_maps


def kernel(cls_score, labels):
    nc = _get_nc()
    in_maps = make_in_maps(cls_score, labels)
    res = run_bass_kernel_spmd(nc, in_maps, core_ids=list(range(N_CORES)))
    total = np.sum(
        [r["out"].astype(np.float64).sum() for r in res.results]
    )
    return np.float32(-(total / N))


# revision 8
# speedup vs baseline: 1.0836x; 1.0836x over previous
"""ArcFace (AngularPenaltySMLoss) over [32768, 8192] f32, distributed over
8 TRN2 NeuronCores, data-parallel on the batch dim.

Per core: shard [4096, 8192]. For each 128-row tile:
  - DMA tile to SBUF (sync-engine HWDGE ring, dedicated to the x stream)
  - ScalarE: exp(S*x) with fused free-dim accumulation -> row exp-sums
  - VectorE: scalar_tensor_tensor (iota == label) * x with fused free-dim
    accumulation -> gathers target = x[row, label] (one nonzero per row)
Epilogue (two batches; the first hides inside the loop):
  numerator = S*(t*cos(M) - sin(M)*sqrt(1 - t^2))   # = S*cos(acos(t)+M)
  with sqrt(y) computed as exp(0.5*ln(y)) so the only ACT table set used
  anywhere is natural_log_exp (zero mid-kernel table switches).
  L = numerator - log(exp(numerator) + rowsum - exp(S*t))
  partial = sum(L) per core -> [128,1]; host sums, loss = -total/N.
"""

import numpy as np

from concourse import bacc, mybir, tile
from concourse.bass_utils import run_bass_kernel_spmd

N, C = 32768, 8192
N_CORES = 8
N_SHARD = N // N_CORES      # 4096 rows per core
P = 128                     # SBUF partitions
N_TILES = N_SHARD // P      # 32 tiles per core
S = 32.0
M = 0.5
EPS = 1e-7

_F32 = mybir.dt.float32


def build(n_shard=N_SHARD, c=C):
    n_tiles = n_shard // P
    nc = bacc.Bacc(None, target_bir_lowering=False)

    x_ext = nc.declare_dram_parameter("cls_score", [n_shard, c], _F32, isOutput=False)
    lab_ext = nc.declare_dram_parameter("labels_t", [P, n_tiles], _F32, isOutput=False)
    out_ext = nc.declare_dram_parameter("out", [P, 1], _F32, isOutput=True)

    AF = mybir.ActivationFunctionType
    OP = mybir.AluOpType
    AX = mybir.AxisListType

    split = max(n_tiles - 8, 1) if n_tiles > 1 else 1

    with tile.TileContext(nc) as tc:
        with (
            tc.tile_pool(name="xp", bufs=2) as xp,
            tc.tile_pool(name="ep", bufs=1) as ep,
            tc.tile_pool(name="mp", bufs=1) as mp,
            tc.tile_pool(name="st", bufs=1) as st,
        ):
            lab = st.tile([P, n_tiles], _F32)
            nc.scalar.dma_start(out=lab[:], in_=lab_ext[:])
            iota = st.tile([P, c], _F32)  # each row = [0..c-1]
            nc.gpsimd.iota(iota[:], pattern=[[1, c]], base=0,
                           channel_multiplier=0,
                           allow_small_or_imprecise_dtypes=True)

            sumexp = st.tile([P, n_tiles], _F32)
            tvals = st.tile([P, n_tiles], _F32)

            # epilogue scratch, written in column batches
            tclip = st.tile([P, n_tiles], _F32)
            tsq = st.tile([P, n_tiles], _F32)
            om = st.tile([P, n_tiles], _F32)
            lnom = st.tile([P, n_tiles], _F32)
            r = st.tile([P, n_tiles], _F32)
            b_t = st.tile([P, n_tiles], _F32)
            num = st.tile([P, n_tiles], _F32)
            e_num = st.tile([P, n_tiles], _F32)
            e_st = st.tile([P, n_tiles], _F32)
            excl = st.tile([P, n_tiles], _F32)
            denom = st.tile([P, n_tiles], _F32)
            logd = st.tile([P, n_tiles], _F32)
            ell = st.tile([P, n_tiles], _F32)

            def epilogue(sl):
                # all [P, width] ops; only Exp/Ln on ACT (one table set)
                nc.vector.tensor_scalar(
                    tclip[:, sl], tvals[:, sl], -1.0 + EPS, 1.0 - EPS,
                    OP.max, OP.min)
                nc.vector.tensor_tensor(tsq[:, sl], tclip[:, sl], tclip[:, sl],
                                        OP.mult)
                nc.vector.tensor_scalar(om[:, sl], tsq[:, sl], -1.0, 1.0,
                                        OP.mult, OP.add)  # 1 - t^2
                nc.scalar.activation(out=lnom[:, sl], in_=om[:, sl], func=AF.Ln)
                nc.scalar.activation(out=r[:, sl], in_=lnom[:, sl], func=AF.Exp,
                                     scale=0.5)  # sqrt(1-t^2)
                nc.vector.tensor_scalar_mul(b_t[:, sl], r[:, sl],
                                            S * float(np.sin(M)))
                nc.vector.scalar_tensor_tensor(
                    num[:, sl], tclip[:, sl], S * float(np.cos(M)), b_t[:, sl],
                    OP.mult, OP.subtract)
                nc.scalar.activation(out=e_num[:, sl], in_=num[:, sl], func=AF.Exp)
                nc.scalar.activation(out=e_st[:, sl], in_=tvals[:, sl],
                                     func=AF.Exp, scale=S)
                nc.vector.scalar_tensor_tensor(
                    excl[:, sl], e_st[:, sl], -1.0, sumexp[:, sl],
                    OP.mult, OP.add)  # sumexp - exp(S t)
                nc.vector.tensor_tensor(denom[:, sl], excl[:, sl], e_num[:, sl],
                                        OP.add)
                nc.scalar.activation(out=logd[:, sl], in_=denom[:, sl], func=AF.Ln)
                nc.vector.tensor_tensor(ell[:, sl], num[:, sl], logd[:, sl],
                                        OP.subtract)

            for k in range(n_tiles):
                xt = xp.tile([P, c], _F32)
                nc.sync.dma_start(out=xt[:], in_=x_ext[k * P:(k + 1) * P, :])
                et = ep.tile([P, c], _F32)
                nc.scalar.activation(
                    out=et[:], in_=xt[:], func=AF.Exp, scale=S,
                    accum_out=sumexp[:, k:k + 1],
                )
                mt = mp.tile([P, c], _F32)
                # mt = (iota == label) * x ; accum -> x[row, label]
                nc.vector.scalar_tensor_tensor(
                    mt[:], iota[:], lab[:, k:k + 1], xt[:],
                    OP.is_equal, OP.mult,
                    accum_out=tvals[:, k:k + 1],
                )
                if k == split - 1 and n_tiles > 1:
                    epilogue(slice(0, split))

            epilogue(slice(split, n_tiles) if n_tiles > 1 else slice(0, n_tiles))

            lrow = st.tile([P, 1], _F32)
            nc.vector.tensor_reduce(lrow[:], ell[:], axis=AX.X, op=OP.add)
            nc.scalar.dma_start(out=out_ext[:], in_=lrow[:])

    nc.finalize()
    return nc


_NC_CACHE = {}


def _get_nc():
    if "nc" not in _NC_CACHE:
        _NC_CACHE["nc"] = build()
    return _NC_CACHE["nc"]


def make_in_maps(cls_score, labels):
    cls_score = np.ascontiguousarray(np.asarray(cls_score, dtype=np.float32))
    labels = np.asarray(labels).astype(np.int64)
    in_maps = []
    for i in range(N_CORES):
        shard = cls_score[i * N_SHARD:(i + 1) * N_SHARD]
        lab_i = labels[i * N_SHARD:(i + 1) * N_SHARD].astype(np.float32)
        # [n_tiles, P] -> [P, n_tiles]: partition p, col k = label of row k*P+p
        lab_t = np.ascontiguousarray(lab_i.reshape(N_TILES, P).T)
        in_maps.append({"cls_score": shard, "labels_t": lab_t})
    return in_maps


def kernel(cls_score, labels):
    nc = _get_nc()
    in_maps = make_in_maps(cls_score, labels)
    res = run_bass_kernel_spmd(nc, in_maps, core_ids=list(range(N_CORES)))
    total = np.sum(
        [r["out"].astype(np.float64).sum() for r in res.results]
    )
    return np.float32(-(total / N))
